# revision 21
# baseline (speedup 1.0000x reference)
"""Bahdanau additive attention on Trainium2 (Bass/Tile), SPMD over 8 NeuronCores.

Problem: attn_out[b,t,:] = softmax_s(v . tanh(enc_f[b,s,:] + qry_f[b,t,:])) @ enc[b]
  with enc_f = enc @ W_h^T, qry_f = q @ W_s^T, masked to s < src_lengths[b].

Sharding: parallel over tgt_len T — core i handles query rows [i*32,(i+1)*32)
for ALL batches; weights and encoder outputs replicated. Every core does the
same amount of work (all batches, full valid src range) so load is perfectly
balanced despite variable src_lengths.

Key layout trick: keep the hidden dim h on SBUF partitions. Then for a fixed
query row t, tanh(enc_fT[h,s] + qry_fT[h,t]) is a per-partition broadcast add
(DVE tensor_scalar, 4x bf16 mode) followed by one big ACT Tanh, and the
reduction over h is an M=1 matmul with v as the stationary operand. Col-tiled
matmuls (tile_position) place 4 query rows per PSUM bank group so 16 rows
accumulate per sweep; a PSUM->SBUF copy + SBUF->SBUF DMA gathers them into
[16, L] softmax layout.

src_lengths are read on the host at trace time: all loop extents are
specialized to L_b (padded up to a multiple of 4); masked source positions are
never computed.
"""

import math
import os

import numpy as np

NCORES = 8
P = 128


def _build_program(B, T_core, S, H, L, Lh):
    import concourse.bass as bass  # noqa: F401
    import concourse.mybir as mybir
    import concourse.tile as tile
    from concourse import bacc
    from concourse.masks import make_identity

    f32 = mybir.dt.float32
    f32r = mybir.dt.float32r
    bf16 = mybir.dt.bfloat16
    AF = mybir.ActivationFunctionType

    HC = H // P  # h chunks (4)

    # Bacc (not raw Bass): its compile() legalizes sync waits (matmuls can
    # carry at most one wait in hardware; extra waits move to ldweights /
    # event semaphores).
    nc = bacc.Bacc("TRN2", target_bir_lowering=False, debug=False)

    enc_d = nc.declare_dram_parameter("enc", [B, S, H], f32, isOutput=False)
    q_d = nc.declare_dram_parameter("q", [B, T_core, H], f32, isOutput=False)
    wh_d = nc.declare_dram_parameter("wh", [H, H], f32, isOutput=False)
    ws_d = nc.declare_dram_parameter("ws", [H, H], f32, isOutput=False)
    v_d = nc.declare_dram_parameter("v", [H], f32, isOutput=False)
    out_d = nc.declare_dram_parameter("out", [B, T_core, H], f32, isOutput=True)

    def r32(ap):
        return ap.bitcast(f32r)

    with tile.TileContext(nc) as tc:
        with (
            tc.tile_pool(name="const", bufs=1) as constp,
            tc.tile_pool(name="sb", bufs=2) as sb,
            tc.tile_pool(name="work", bufs=2) as workp,
            tc.tile_pool(name="ps", bufs=2, space="PSUM") as psp,
            tc.tile_pool(name="ps_sc", bufs=1, space="PSUM") as pssc,
        ):
            ident_f = constp.tile([P, P], f32)
            make_identity(nc, ident_f)
            ident_b = constp.tile([P, P], bf16)
            make_identity(nc, ident_b)

            # v -> [128, HC] f32 -> bf16 (column c = chunk c of v)
            v_f = constp.tile([P, HC], f32)
            nc.sync.dma_start(v_f, v_d.rearrange("(c p) -> p c", p=P))
            v_bf = constp.tile([P, HC], bf16)
            nc.vector.tensor_copy(v_bf, v_f)
            # v32 block c = [v chunk c, 0 x 31]: M=32 score matmuls write all
            # 32 partitions of a col-group (rows 1..31 = 0) so PSUM reads are
            # fully initialized; cost is unchanged (matmul time ~ N only).
            v32 = constp.tile([P, HC * 32], bf16)
            nc.vector.memset(v32, 0.0)
            for c in range(HC):
                nc.vector.tensor_copy(v32[:, c * 32 : c * 32 + 1], v_f[:, c : c + 1])

            # W_h^T in bf16, W_s^T in f32.
            # whT block k (cols [k*H,(k+1)*H)) = W_h^T[h' in chunk k, :]
            whT = constp.tile([P, HC * H], bf16)
            wsT = constp.tile([P, HC * H], f32)
            whnb = []
            wsn = []
            for c in range(HC):
                wn = sb.tile([P, H], f32, name=f"whn{c}", tag="wn", bufs=4)
                nc.sync.dma_start(wn, wh_d[c * P : (c + 1) * P, :])
                wnb = sb.tile([P, H], bf16, name=f"whnb{c}", tag="wnb", bufs=4)
                nc.vector.tensor_copy(wnb, wn)
                whnb.append(wnb)
                wsn_c = sb.tile([P, H], f32, name=f"wsn{c}", tag="wsn", bufs=4)
                nc.sync.dma_start(wsn_c, ws_d[c * P : (c + 1) * P, :])
                wsn.append(wsn_c)
            for k in range(HC):
                pst = psp.tile([P, HC * P], bf16, name=f"whT_ps{k}", tag="mm")
                for c in range(HC):
                    nc.tensor.transpose(
                        pst[:, c * P : (c + 1) * P],
                        whnb[c][:, k * P : (k + 1) * P],
                        ident_b,
                    )
                nc.vector.tensor_copy(whT[:, k * H : (k + 1) * H], pst)
            for k in range(HC):
                psf = psp.tile([P, HC * P], f32, name=f"wsT_ps{k}", tag="mm")
                for c in range(HC):
                    nc.tensor.transpose(
                        psf[:, c * P : (c + 1) * P],
                        wsn[c][:, k * P : (k + 1) * P],
                        ident_f,
                    )
                nc.vector.tensor_copy(wsT[:, k * H : (k + 1) * H], psf)

            for b in range(B):
                Lb, Lhb = L[b], Lh[b]
                nk = (Lhb + P - 1) // P  # source chunks (1 or 2)

                # ---- load encoder rows (only the valid s range) ----
                enc_nat = []
                for k2 in range(nk):
                    r2 = min(P, Lhb - k2 * P)
                    en = sb.tile([P, H], f32, name=f"enc{b}_{k2}", tag=f"enc{k2}")
                    nc.sync.dma_start(
                        en[:r2, :], enc_d[b, k2 * P : k2 * P + r2, :]
                    )
                    enc_nat.append((en, r2))

                # ---- encT (bf16): block k = enc^T[h' in chunk k, s] ----
                encT = sb.tile([P, HC * S], bf16, name=f"encT{b}", tag="encT")
                encT_v = encT.rearrange("p (k s) -> p k s", k=HC)
                for k2 in range(nk):
                    en, r2 = enc_nat[k2]
                    ps_t = psp.tile([P, HC * P], f32, name=f"encT_ps{b}_{k2}", tag="mm")
                    for k in range(HC):
                        nc.tensor.transpose(
                            ps_t[:, k * P : k * P + r2],
                            en[:r2, k * P : (k + 1) * P],
                            ident_f[:r2, :r2],
                        )
                    nc.vector.tensor_copy(
                        encT_v[:, :, k2 * P : k2 * P + r2],
                        ps_t.rearrange("p (k s) -> p k s", k=HC)[:, :, :r2],
                    )

                # ---- enc_fT (bf16): block c = (W_h @ enc^T)[h in chunk c, s] ----
                ps_e = psp.tile([P, HC * S], f32, name=f"encf_ps{b}", tag="mm")
                for c in range(HC):
                    for k in range(HC):
                        nc.tensor.matmul(
                            ps_e[:, c * S : c * S + Lhb],
                            whT[:, k * H + c * P : k * H + (c + 1) * P],
                            encT_v[:, k, :Lhb],
                            start=(k == 0),
                            stop=(k == HC - 1),
                        )
                encfT = sb.tile([P, HC * S], bf16, name=f"encfT{b}", tag="encfT")
                nc.vector.tensor_copy(
                    encfT.rearrange("p (c s) -> p c s", c=HC)[:, :, :Lhb],
                    ps_e.rearrange("p (c s) -> p c s", c=HC)[:, :, :Lhb],
                )

                # ---- qry_fT (f32): block c cols = (W_s @ q^T)[h in chunk c, t] ----
                qn = sb.tile([T_core, H], f32, name=f"qn{b}", tag="qn")
                nc.sync.dma_start(qn, q_d[b])
                ps_q = psp.tile([P, HC * T_core], f32, name=f"qT_ps{b}", tag="mm")
                for k in range(HC):
                    nc.tensor.transpose(
                        ps_q[:, k * T_core : (k + 1) * T_core],
                        qn[:, k * P : (k + 1) * P],
                        ident_f[:T_core, :T_core],
                    )
                qT = sb.tile([P, HC * T_core], f32, name=f"qT{b}", tag="qT")
                nc.vector.tensor_copy(qT, ps_q)
                ps_qf = psp.tile([P, HC * T_core], f32, name=f"qf_ps{b}", tag="mm")
                for c in range(HC):
                    for k in range(HC):
                        nc.tensor.matmul(
                            ps_qf[:, c * T_core : (c + 1) * T_core],
                            wsT[:, k * H + c * P : k * H + (c + 1) * P],
                            qT[:, k * T_core : (k + 1) * T_core],
                            start=(k == 0),
                            stop=(k == HC - 1),
                        )
                qfT = sb.tile([P, HC * T_core], f32, name=f"qfT{b}", tag="qfT")
                nc.vector.tensor_copy(qfT, ps_qf)

                # ---- scores: sweeps of 16 query rows ----
                n_sweeps = T_core // 16
                scores_sw = []
                for sweep in range(n_sweeps):
                    ps_scores = pssc.tile(
                        [P, 4 * 512], f32, name=f"sc_ps{b}_{sweep}", tag="scores"
                    )
                    tanh_tiles = []
                    for c in range(HC):
                        sum_t = workp.tile(
                            [P, 16 * Lhb], bf16, name=f"sum{b}_{sweep}_{c}",
                            tag="sum", bufs=3,
                        )
                        for tt in range(16):
                            tg = sweep * 16 + tt
                            nc.vector.tensor_scalar_add(
                                sum_t[:, tt * Lhb : (tt + 1) * Lhb],
                                encfT[:, c * S : c * S + Lhb],
                                qfT[:, c * T_core + tg : c * T_core + tg + 1],
                            )
                        tanh_t = workp.tile(
                            [P, 16 * Lhb], bf16, name=f"tanh{b}_{sweep}_{c}",
                            tag="tanh", bufs=5,
                        )
                        nc.scalar.activation(tanh_t, sum_t, AF.Tanh)
                        tanh_tiles.append(tanh_t)
                    # per query row: 4 consecutive chunk-matmuls (no group
                    # interleaving within a PSUM bank)
                    for tt in range(16):
                        cg, j = tt // 4, tt % 4  # row group = 32*cg, bank = j
                        for c in range(HC):
                            nc.tensor.matmul(
                                ps_scores[32 * cg : 32 * cg + 32, 512 * j : 512 * j + Lhb],
                                v32[:, c * 32 : (c + 1) * 32],
                                tanh_tiles[c][:, tt * Lhb : (tt + 1) * Lhb],
                                start=(c == 0),
                                stop=(c == HC - 1),
                                tile_position=(0, 32 * cg),
                            )
                    # PSUM -> SBUF staging copy (partition-preserving), then
                    # a gather DMA to softmax layout: t_local = 4*cg + j
                    stage = sb.tile(
                        [P, 4 * S], f32, name=f"stage{b}_{sweep}", tag="stage"
                    )
                    nc.vector.tensor_copy(
                        stage.rearrange("p (j s) -> p j s", j=4)[:, :, :Lb],
                        ps_scores.rearrange("p (j s) -> p j s", j=4)[:, :, :Lb],
                    )
                    sc_w = sb.tile(
                        [16, S], f32, name=f"scores{b}_{sweep}", tag="scsb"
                    )
                    src = stage.rearrange("(a p) (j s) -> a p j s", a=4, j=4)[
                        :, 0, :, :Lb
                    ]
                    nc.sync.dma_start(sc_w[:, :Lb], src)
                    scores_sw.append(sc_w)

                # ---- per-sweep softmax + attention (engine ops need 32-aligned
                # partition bases, so 16-row sweeps each live at partition 0) ----
                for sweep in range(n_sweeps):
                    sc_w = scores_sw[sweep]
                    negmax = sb.tile(
                        [16, 1], f32, name=f"negmax{b}_{sweep}", tag="negmax"
                    )
                    nc.vector.tensor_reduce(
                        negmax, sc_w[:, :Lb], axis=mybir.AxisListType.X,
                        op=mybir.AluOpType.max, negate=True,
                    )
                    w_sw = sb.tile([16, S], f32, name=f"w{b}_{sweep}", tag="w")
                    if Lb < S:
                        nc.vector.memset(w_sw[:, Lb:], 0.0)
                    sums = sb.tile([16, 1], f32, name=f"sums{b}_{sweep}", tag="sums")
                    nc.scalar.activation(
                        w_sw[:, :Lb], sc_w[:, :Lb],
                        AF.Exp, bias=negmax, accum_out=sums,
                    )
                    recip = sb.tile(
                        [16, 1], f32, name=f"recip{b}_{sweep}", tag="recip"
                    )
                    nc.vector.reciprocal(recip, sums)

                    # attn_out = (w_raw @ enc) * recip
                    ps_w = psp.tile([P, 2 * 16], f32, name=f"wT_ps{b}_{sweep}", tag="mm")
                    for k2 in range(nk):
                        nc.tensor.transpose(
                            ps_w[:, k2 * 16 : (k2 + 1) * 16],
                            w_sw[:, k2 * P : (k2 + 1) * P],
                            ident_f[:16, :16],
                        )
                    wT = sb.tile([P, 2 * 16], f32, name=f"wT{b}_{sweep}", tag="wT")
                    nc.vector.tensor_copy(
                        wT[:, : nk * 16], ps_w[:, : nk * 16]
                    )
                    ps_attn = psp.tile([16, H], f32, name=f"attn_ps{b}_{sweep}", tag="mm")
                    for k2 in range(nk):
                        en, r2 = enc_nat[k2]
                        nc.tensor.matmul(
                            ps_attn,
                            wT[:r2, k2 * 16 : (k2 + 1) * 16],
                            en[:r2, :],
                            start=(k2 == 0),
                            stop=(k2 == nk - 1),
                        )
                    out_sb = sb.tile([16, H], f32, name=f"out{b}_{sweep}", tag="outsb")
                    nc.vector.tensor_scalar_mul(out_sb, ps_attn, recip)
                    nc.sync.dma_start(
                        out_d[b, sweep * 16 : (sweep + 1) * 16, :], out_sb
                    )

    nc.compile()
    return nc


LAST_EXEC_NS = None


def _get_program(key):
    # Build fresh every time: lowering through bass2jax mutates the nc
    # (partition-id preamble), so an nc must not be lowered twice.
    B, T_core, S, H, L, Lh = key
    return _build_program(B, T_core, S, H, list(L), list(Lh))


def kernel(query, encoder_outputs, src_lengths, W_h, W_s, v):
    global LAST_EXEC_NS
    from concourse.bass_utils import run_bass_kernel_spmd

    query = np.ascontiguousarray(np.asarray(query, dtype=np.float32))
    enc = np.ascontiguousarray(np.asarray(encoder_outputs, dtype=np.float32))
    W_h = np.ascontiguousarray(np.asarray(W_h, dtype=np.float32))
    W_s = np.ascontiguousarray(np.asarray(W_s, dtype=np.float32))
    v = np.ascontiguousarray(np.asarray(v, dtype=np.float32)).reshape(-1)
    L = [int(x) for x in np.asarray(src_lengths).reshape(-1)]

    B, T, H = query.shape
    S = enc.shape[1]
    T_core = T // NCORES
    Lh = [min(S, ((l + 3) // 4) * 4) for l in L]

    nc = _get_program((B, T_core, S, H, tuple(L), tuple(Lh)))

    in_maps = [
        {
            "enc": enc,
            "q": np.ascontiguousarray(query[:, i * T_core : (i + 1) * T_core, :]),
            "wh": W_h,
            "ws": W_s,
            "v": v,
        }
        for i in range(NCORES)
    ]
    res = run_bass_kernel_spmd(nc, in_maps, list(range(NCORES)))
    LAST_EXEC_NS = res.exec_time_ns
    out = np.concatenate([res.results[i]["out"] for i in range(NCORES)], axis=1)
    return out


# revision 24
# speedup vs baseline: 14.8074x; 14.8074x over previous
"""Bahdanau additive attention on Trainium2 (Bass/Tile), SPMD over 8 NeuronCores.

Problem: attn_out[b,t,:] = softmax_s(v . tanh(enc_f[b,s,:] + qry_f[b,t,:])) @ enc[b]
  with enc_f = enc @ W_h^T, qry_f = q @ W_s^T, masked to s < src_lengths[b].

Sharding: parallel over tgt_len T — core i handles query rows [i*32,(i+1)*32)
for ALL batches; weights and encoder outputs replicated. Every core does the
same amount of work (all batches, full valid src range) so load is perfectly
balanced despite variable src_lengths.

Key layout trick: keep the hidden dim h on SBUF partitions. Then for a fixed
query row t, tanh(enc_fT[h,s] + qry_fT[h,t]) is a per-partition broadcast add
(DVE tensor_scalar, 4x bf16 mode) followed by one big ACT Tanh, and the
reduction over h is an M=1 matmul with v as the stationary operand. Col-tiled
matmuls (tile_position) place 4 query rows per PSUM bank group so 16 rows
accumulate per sweep; a PSUM->SBUF copy + SBUF->SBUF DMA gathers them into
[16, L] softmax layout.

src_lengths are read on the host at trace time: all loop extents are
specialized to L_b (padded up to a multiple of 4); masked source positions are
never computed.
"""

import math
import os

import numpy as np

NCORES = 8
P = 128


def _build_program(B, T_core, S, H, L, Lh, reps=1):
    import concourse.bass as bass  # noqa: F401
    import concourse.mybir as mybir
    import concourse.tile as tile
    from concourse import bacc
    from concourse.masks import make_identity

    f32 = mybir.dt.float32
    f32r = mybir.dt.float32r
    bf16 = mybir.dt.bfloat16
    AF = mybir.ActivationFunctionType

    HC = H // P  # h chunks (4)

    # Bacc (not raw Bass): its compile() legalizes sync waits (matmuls can
    # carry at most one wait in hardware; extra waits move to ldweights /
    # event semaphores).
    nc = bacc.Bacc("TRN2", target_bir_lowering=False, debug=False)

    enc_d = nc.declare_dram_parameter("enc", [B, S, H], f32, isOutput=False)
    q_d = nc.declare_dram_parameter("q", [B, T_core, H], f32, isOutput=False)
    wh_d = nc.declare_dram_parameter("wh", [H, H], f32, isOutput=False)
    ws_d = nc.declare_dram_parameter("ws", [H, H], f32, isOutput=False)
    v_d = nc.declare_dram_parameter("v", [H], f32, isOutput=False)
    out_d = nc.declare_dram_parameter("out", [B, T_core, H], f32, isOutput=True)

    def r32(ap):
        return ap.bitcast(f32r)

    with tile.TileContext(nc) as tc:
        with (
            tc.tile_pool(name="const", bufs=1) as constp,
            tc.tile_pool(name="sb", bufs=2) as sb,
            tc.tile_pool(name="work", bufs=2) as workp,
            tc.tile_pool(name="ps", bufs=2, space="PSUM") as psp,
            tc.tile_pool(name="ps_sc", bufs=1, space="PSUM") as pssc,
        ):
            ident_f = constp.tile([P, P], f32)
            make_identity(nc, ident_f)
            ident_b = constp.tile([P, P], bf16)
            make_identity(nc, ident_b)

            # v -> [128, HC] f32 -> bf16 (column c = chunk c of v)
            v_f = constp.tile([P, HC], f32)
            nc.sync.dma_start(v_f, v_d.rearrange("(c p) -> p c", p=P))
            v_bf = constp.tile([P, HC], bf16)
            nc.vector.tensor_copy(v_bf, v_f)
            # v32 block c = [v chunk c, 0 x 31]: M=32 score matmuls write all
            # 32 partitions of a col-group (rows 1..31 = 0) so PSUM reads are
            # fully initialized; cost is unchanged (matmul time ~ N only).
            v32 = constp.tile([P, HC * 32], bf16)
            nc.vector.memset(v32, 0.0)
            for c in range(HC):
                nc.vector.tensor_copy(v32[:, c * 32 : c * 32 + 1], v_f[:, c : c + 1])

            # W_h^T in bf16, W_s^T in f32.
            # whT block k (cols [k*H,(k+1)*H)) = W_h^T[h' in chunk k, :]
            whT = constp.tile([P, HC * H], bf16)
            wsT = constp.tile([P, HC * H], f32)
            whnb = []
            wsn = []
            for c in range(HC):
                wn = sb.tile([P, H], f32, name=f"whn{c}", tag="wn", bufs=4)
                nc.sync.dma_start(wn, wh_d[c * P : (c + 1) * P, :])
                wnb = sb.tile([P, H], bf16, name=f"whnb{c}", tag="wnb", bufs=4)
                nc.vector.tensor_copy(wnb, wn)
                whnb.append(wnb)
                wsn_c = sb.tile([P, H], f32, name=f"wsn{c}", tag="wsn", bufs=4)
                nc.sync.dma_start(wsn_c, ws_d[c * P : (c + 1) * P, :])
                wsn.append(wsn_c)
            for k in range(HC):
                pst = psp.tile([P, HC * P], bf16, name=f"whT_ps{k}", tag="mm")
                for c in range(HC):
                    nc.tensor.transpose(
                        pst[:, c * P : (c + 1) * P],
                        whnb[c][:, k * P : (k + 1) * P],
                        ident_b,
                    )
                nc.vector.tensor_copy(whT[:, k * H : (k + 1) * H], pst)
            for k in range(HC):
                psf = psp.tile([P, HC * P], f32, name=f"wsT_ps{k}", tag="mm")
                for c in range(HC):
                    nc.tensor.transpose(
                        psf[:, c * P : (c + 1) * P],
                        wsn[c][:, k * P : (k + 1) * P],
                        ident_f,
                    )
                nc.vector.tensor_copy(wsT[:, k * H : (k + 1) * H], psf)

            def batch_loop():
              for b in range(B):
                Lb, Lhb = L[b], Lh[b]
                nk = (Lhb + P - 1) // P  # source chunks (1 or 2)

                # ---- load encoder rows (only the valid s range) ----
                enc_nat = []
                for k2 in range(nk):
                    r2 = min(P, Lhb - k2 * P)
                    en = sb.tile([P, H], f32, name=f"enc{b}_{k2}", tag=f"enc{k2}")
                    nc.sync.dma_start(
                        en[:r2, :], enc_d[b, k2 * P : k2 * P + r2, :]
                    )
                    enc_nat.append((en, r2))

                # ---- encT (bf16): block k = enc^T[h' in chunk k, s] ----
                encT = sb.tile([P, HC * S], bf16, name=f"encT{b}", tag="encT")
                encT_v = encT.rearrange("p (k s) -> p k s", k=HC)
                for k2 in range(nk):
                    en, r2 = enc_nat[k2]
                    ps_t = psp.tile([P, HC * P], f32, name=f"encT_ps{b}_{k2}", tag="mm")
                    for k in range(HC):
                        nc.tensor.transpose(
                            ps_t[:, k * P : k * P + r2],
                            en[:r2, k * P : (k + 1) * P],
                            ident_f[:r2, :r2],
                        )
                    nc.vector.tensor_copy(
                        encT_v[:, :, k2 * P : k2 * P + r2],
                        ps_t.rearrange("p (k s) -> p k s", k=HC)[:, :, :r2],
                    )

                # ---- enc_fT (bf16): block c = (W_h @ enc^T)[h in chunk c, s] ----
                ps_e = psp.tile([P, HC * S], f32, name=f"encf_ps{b}", tag="mm")
                for c in range(HC):
                    for k in range(HC):
                        nc.tensor.matmul(
                            ps_e[:, c * S : c * S + Lhb],
                            whT[:, k * H + c * P : k * H + (c + 1) * P],
                            encT_v[:, k, :Lhb],
                            start=(k == 0),
                            stop=(k == HC - 1),
                        )
                encfT = sb.tile([P, HC * S], bf16, name=f"encfT{b}", tag="encfT")
                nc.vector.tensor_copy(
                    encfT.rearrange("p (c s) -> p c s", c=HC)[:, :, :Lhb],
                    ps_e.rearrange("p (c s) -> p c s", c=HC)[:, :, :Lhb],
                )

                # ---- qry_fT (f32): block c cols = (W_s @ q^T)[h in chunk c, t] ----
                qn = sb.tile([T_core, H], f32, name=f"qn{b}", tag="qn")
                nc.sync.dma_start(qn, q_d[b])
                ps_q = psp.tile([P, HC * T_core], f32, name=f"qT_ps{b}", tag="mm")
                for k in range(HC):
                    nc.tensor.transpose(
                        ps_q[:, k * T_core : (k + 1) * T_core],
                        qn[:, k * P : (k + 1) * P],
                        ident_f[:T_core, :T_core],
                    )
                qT = sb.tile([P, HC * T_core], f32, name=f"qT{b}", tag="qT")
                nc.vector.tensor_copy(qT, ps_q)
                ps_qf = psp.tile([P, HC * T_core], f32, name=f"qf_ps{b}", tag="mm")
                for c in range(HC):
                    for k in range(HC):
                        nc.tensor.matmul(
                            ps_qf[:, c * T_core : (c + 1) * T_core],
                            wsT[:, k * H + c * P : k * H + (c + 1) * P],
                            qT[:, k * T_core : (k + 1) * T_core],
                            start=(k == 0),
                            stop=(k == HC - 1),
                        )
                qfT = sb.tile([P, HC * T_core], f32, name=f"qfT{b}", tag="qfT")
                nc.vector.tensor_copy(qfT, ps_qf)

                # ---- scores: sweeps of 16 query rows ----
                n_sweeps = T_core // 16
                scores_sw = []
                for sweep in range(n_sweeps):
                    ps_scores = pssc.tile(
                        [P, 4 * 512], f32, name=f"sc_ps{b}_{sweep}", tag="scores"
                    )
                    tanh_tiles = []
                    for c in range(HC):
                        sum_t = workp.tile(
                            [P, 16 * Lhb], bf16, name=f"sum{b}_{sweep}_{c}",
                            tag="sum", bufs=3,
                        )
                        for tt in range(16):
                            tg = sweep * 16 + tt
                            nc.vector.tensor_scalar_add(
                                sum_t[:, tt * Lhb : (tt + 1) * Lhb],
                                encfT[:, c * S : c * S + Lhb],
                                qfT[:, c * T_core + tg : c * T_core + tg + 1],
                            )
                        tanh_t = workp.tile(
                            [P, 16 * Lhb], bf16, name=f"tanh{b}_{sweep}_{c}",
                            tag="tanh", bufs=5,
                        )
                        nc.scalar.activation(tanh_t, sum_t, AF.Tanh)
                        tanh_tiles.append(tanh_t)
                    # per query row: 4 consecutive chunk-matmuls (no group
                    # interleaving within a PSUM bank)
                    for tt in range(16):
                        cg, j = tt // 4, tt % 4  # row group = 32*cg, bank = j
                        for c in range(HC):
                            nc.tensor.matmul(
                                ps_scores[32 * cg : 32 * cg + 32, 512 * j : 512 * j + Lhb],
                                v32[:, c * 32 : (c + 1) * 32],
                                tanh_tiles[c][:, tt * Lhb : (tt + 1) * Lhb],
                                start=(c == 0),
                                stop=(c == HC - 1),
                                tile_position=(0, 32 * cg),
                            )
                    # PSUM -> SBUF staging copy (partition-preserving), then
                    # a gather DMA to softmax layout: t_local = 4*cg + j
                    stage = sb.tile(
                        [P, 4 * S], f32, name=f"stage{b}_{sweep}", tag="stage"
                    )
                    nc.vector.tensor_copy(
                        stage.rearrange("p (j s) -> p j s", j=4)[:, :, :Lb],
                        ps_scores.rearrange("p (j s) -> p j s", j=4)[:, :, :Lb],
                    )
                    sc_w = sb.tile(
                        [16, S], f32, name=f"scores{b}_{sweep}", tag="scsb"
                    )
                    src = stage.rearrange("(a p) (j s) -> a p j s", a=4, j=4)[
                        :, 0, :, :Lb
                    ]
                    nc.sync.dma_start(sc_w[:, :Lb], src)
                    scores_sw.append(sc_w)

                # ---- per-sweep softmax + attention (engine ops need 32-aligned
                # partition bases, so 16-row sweeps each live at partition 0) ----
                for sweep in range(n_sweeps):
                    sc_w = scores_sw[sweep]
                    negmax = sb.tile(
                        [16, 1], f32, name=f"negmax{b}_{sweep}", tag="negmax"
                    )
                    nc.vector.tensor_reduce(
                        negmax, sc_w[:, :Lb], axis=mybir.AxisListType.X,
                        op=mybir.AluOpType.max, negate=True,
                    )
                    w_sw = sb.tile([16, S], f32, name=f"w{b}_{sweep}", tag="w")
                    if Lb < S:
                        nc.vector.memset(w_sw[:, Lb:], 0.0)
                    sums = sb.tile([16, 1], f32, name=f"sums{b}_{sweep}", tag="sums")
                    nc.scalar.activation(
                        w_sw[:, :Lb], sc_w[:, :Lb],
                        AF.Exp, bias=negmax, accum_out=sums,
                    )
                    recip = sb.tile(
                        [16, 1], f32, name=f"recip{b}_{sweep}", tag="recip"
                    )
                    nc.vector.reciprocal(recip, sums)

                    # attn_out = (w_raw @ enc) * recip
                    ps_w = psp.tile([P, 2 * 16], f32, name=f"wT_ps{b}_{sweep}", tag="mm")
                    for k2 in range(nk):
                        nc.tensor.transpose(
                            ps_w[:, k2 * 16 : (k2 + 1) * 16],
                            w_sw[:, k2 * P : (k2 + 1) * P],
                            ident_f[:16, :16],
                        )
                    wT = sb.tile([P, 2 * 16], f32, name=f"wT{b}_{sweep}", tag="wT")
                    nc.vector.tensor_copy(
                        wT[:, : nk * 16], ps_w[:, : nk * 16]
                    )
                    ps_attn = psp.tile([16, H], f32, name=f"attn_ps{b}_{sweep}", tag="mm")
                    for k2 in range(nk):
                        en, r2 = enc_nat[k2]
                        nc.tensor.matmul(
                            ps_attn,
                            wT[:r2, k2 * 16 : (k2 + 1) * 16],
                            en[:r2, :],
                            start=(k2 == 0),
                            stop=(k2 == nk - 1),
                        )
                    out_sb = sb.tile([16, H], f32, name=f"out{b}_{sweep}", tag="outsb")
                    nc.vector.tensor_scalar_mul(out_sb, ps_attn, recip)
                    nc.sync.dma_start(
                        out_d[b, sweep * 16 : (sweep + 1) * 16, :], out_sb
                    )

            if reps > 1:
                # device-side repetition loop, used only for timing
                with tc.For_i(0, reps, 1):
                    batch_loop()
            else:
                batch_loop()

    nc.compile()
    return nc


LAST_EXEC_NS = None


def _get_program(key):
    # Build fresh every time: lowering through bass2jax mutates the nc
    # (partition-id preamble), so an nc must not be lowered twice.
    B, T_core, S, H, L, Lh = key
    return _build_program(B, T_core, S, H, list(L), list(Lh))


def kernel(query, encoder_outputs, src_lengths, W_h, W_s, v):
    global LAST_EXEC_NS
    from concourse.bass_utils import run_bass_kernel_spmd

    query = np.ascontiguousarray(np.asarray(query, dtype=np.float32))
    enc = np.ascontiguousarray(np.asarray(encoder_outputs, dtype=np.float32))
    W_h = np.ascontiguousarray(np.asarray(W_h, dtype=np.float32))
    W_s = np.ascontiguousarray(np.asarray(W_s, dtype=np.float32))
    v = np.ascontiguousarray(np.asarray(v, dtype=np.float32)).reshape(-1)
    L = [int(x) for x in np.asarray(src_lengths).reshape(-1)]

    B, T, H = query.shape
    S = enc.shape[1]
    T_core = T // NCORES
    Lh = [min(S, ((l + 3) // 4) * 4) for l in L]

    nc = _get_program((B, T_core, S, H, tuple(L), tuple(Lh)))

    in_maps = [
        {
            "enc": enc,
            "q": np.ascontiguousarray(query[:, i * T_core : (i + 1) * T_core, :]),
            "wh": W_h,
            "ws": W_s,
            "v": v,
        }
        for i in range(NCORES)
    ]
    res = run_bass_kernel_spmd(nc, in_maps, list(range(NCORES)))
    LAST_EXEC_NS = res.exec_time_ns
    out = np.concatenate([res.results[i]["out"] for i in range(NCORES)], axis=1)
    return out
